# revision 4
# baseline (speedup 1.0000x reference)
"""Causal multi-head attention (B=2, T=2048, C=1024, H=16, d=64) on 8 trn2 cores.

Sharding: core i -> (batch b = i//4, head group g = i%4, 4 heads/core).
Data parallel over B, tensor parallel over heads; the out-proj partial sums
(contraction over this core's 256 channels) are reduced on the host during
the gather step, along with b_proj and the analytically-folded V bias.

Device kernel works entirely in [feature, token] (transposed) layout so no
on-device transposes are needed. Perf structure (HAM-aware: the PE clock
doubles to 2.4 GHz only under sustained busy, so every stall matters):
  stage 1: Q^T,K^T = (Wqk)^T x^T  with ct-PAIRED psum banks and the k-loop
           innermost+interleaved so matmuls start as soon as each bf16
           x/w k-tile lands (DMA-paced, no dead wait for the full input).
           V = x W_v in natural layout (stage-4 lhsT), ones column appended.
  stage 2: S^T[j,q] per HEAD-PAIR: the two heads of a kT tile live on SBUF
           partitions 0-63 / 64-127, so their K=64 score matmuls land on the
           two 64-row PE tiles and execute CONCURRENTLY when adjacent.
  stage 3: P^T = exp(S^T + mask) on ACT (the binding engine of the attention
           phase); one full-tile exp per group where the stale-psum columns
           are provably never read.
  stage 4: outT[65,q] = [V_h | 1]^T.T @ P^T accumulated over j (row 64 = Z),
           emitted 2 g-cycles behind exp so the in-order PE never waits.
  stage 5: att^T = outT[0:64] * (1/Z) -> bf16, via reciprocal_approx_fast
           (DVE) + partition_broadcast (GPSIMD).
  stage 6: y^T = Wp.T @ att^T (bf16) -> DMA straight out of PSUM; host sums
           partials + transposes.

x / W_qkv / W_v / W_proj travel as bf16 (halves DMA; ~1e-3 rel err), scores
and P stay fp32(r) end-to-end, accumulation always fp32 in PSUM.
"""

import numpy as np
import ml_dtypes

import concourse.bass as bass
import concourse.mybir as mybir
from concourse import bacc
import concourse.tile as tile
from concourse.bass_utils import run_bass_kernel_spmd

B, T, C, H, D = 2, 2048, 1024, 16, 64
NCORES = 8
HPC = 4            # heads per core
CS = HPC * D       # 256 channels per core (per Q/K/V block)
KT = C // 128      # 8 contraction tiles for the projections
NT = T // 128      # 16 token tiles of 128
QB = 512           # query block (psum bank width in fp32)
NQB = T // QB      # 4 query blocks
NEG = -1e9

F32 = mybir.dt.float32
F32R = mybir.dt.float32r
BF16 = mybir.dt.bfloat16

TRACE = False
LAST_RESULT = None


def _build_body(nc, tc, ctx, xT, wqk, wv, bqk, wp, masks, yT):
    AF = mybir.ActivationFunctionType

    persist = ctx.enter_context(tc.tile_pool(name="persist", bufs=1))

    wqk_sb = [persist.tile([128, 2 * CS], BF16, tag=f"wqk{k}", name=f"wqk{k}") for k in range(KT)]
    wv_sb = [persist.tile([128, CS], BF16, tag=f"wv{k}", name=f"wv{k}") for k in range(KT)]
    bqk_sb = [persist.tile([128, 1], F32, tag=f"bqk{c}", name=f"bqk{c}") for c in range(4)]
    wp_sb = [persist.tile([128, C], BF16, tag=f"wp{k}", name=f"wp{k}") for k in range(2)]
    mask_sb = persist.tile([128, 128], F32, tag="mask", name="mask_sb")
    qT_sb = [persist.tile([128, T], F32R, tag=f"qT{i}", name=f"qT{i}") for i in range(2)]
    kT_sb = [persist.tile([128, T], F32R, tag=f"kT{i}", name=f"kT{i}") for i in range(2)]
    v_sb = [persist.tile([128, HPC, D + 1], F32R, tag=f"v{t}", name=f"v{t}") for t in range(NT)]
    attT_sb = [persist.tile([128, T], BF16, tag=f"attT{i}", name=f"attT{i}") for i in range(2)]

    # ---------------- stage 1: projections ----------------
    with (
        tc.tile_pool(name="xpool", bufs=1) as xpool,
        tc.tile_pool(name="s1psum", bufs=8, space="PSUM") as s1p,
    ):
        xT_sb = [xpool.tile([128, T], BF16, tag=f"xT{k}", name=f"xT{k}") for k in range(KT)]
        # DMA order = consumption order: (wqk[k], xT[k]) pairs so the
        # ct-pair k-loop below starts after ~one tile instead of the
        # whole input; wv/wp/masks/bias follow under compute.
        for k in range(KT):
            nc.sync.dma_start(out=wqk_sb[k][:, :], in_=wqk[k * 128:(k + 1) * 128, :])
            nc.sync.dma_start(out=xT_sb[k][:, :], in_=xT[k * 128:(k + 1) * 128, :])
        for c4 in range(4):
            nc.sync.dma_start(out=bqk_sb[c4][:, :], in_=bqk[c4 * 128:(c4 + 1) * 128, :])
        for k in range(KT):
            nc.sync.dma_start(out=wv_sb[k][:, :], in_=wv[k * 128:(k + 1) * 128, :])
        for k in range(2):
            nc.sync.dma_start(out=wp_sb[k][:, :], in_=wp[k * 128:(k + 1) * 128, :])
        nc.sync.dma_start(out=mask_sb[:, :], in_=masks[:, :])
        ones_f32 = xpool.tile([128, 4], F32, tag="ones_f32", name="ones_f32")
        nc.vector.memset(ones_f32[:, :], 1.0)
        for t in range(NT):
            nc.vector.tensor_copy(v_sb[t][:, :, D], ones_f32[:, :])

        # Q^T (ct 0,1) and K^T (ct 2,3): pair (q-heads01, k-heads01) then
        # (q-heads23, k-heads23); 8 live psum banks per pair, k innermost
        # so the PE chews each x k-tile the moment it arrives.
        for cts in ((0, 2), (1, 3)):
            ps = {}
            for ct in cts:
                for tc4 in range(NQB):
                    ps[ct, tc4] = s1p.tile([128, QB], F32, tag="ps", name="ps")
            for k in range(KT):
                for ct in cts:
                    for tc4 in range(NQB):
                        nc.tensor.matmul(
                            ps[ct, tc4][:, :],
                            lhsT=wqk_sb[k][:, ct * 128:(ct + 1) * 128],
                            rhs=xT_sb[k][:, tc4 * QB:(tc4 + 1) * QB],
                            start=(k == 0),
                            stop=(k == KT - 1),
                        )
            for ct in cts:
                dst = qT_sb[ct % 2] if ct < 2 else kT_sb[ct % 2]
                for tc4 in range(NQB):
                    nc.vector.tensor_scalar_add(
                        dst[:, tc4 * QB:(tc4 + 1) * QB], ps[ct, tc4][:, :], bqk_sb[ct][:, :]
                    )

        # V natural layout: V[t, c] over k; ones column appended per head
        for t in range(NT):
            psv = s1p.tile([128, QB], F32, tag="ps", name="ps")
            for k in range(KT):
                nc.tensor.matmul(
                    psv[:, 0:CS],
                    lhsT=xT_sb[k][:, t * 128:(t + 1) * 128],
                    rhs=wv_sb[k][:, :],
                    start=(k == 0),
                    stop=(k == KT - 1),
                )
            nc.any.tensor_copy(
                v_sb[t][:, :, 0:D],
                psv[:, 0:CS].rearrange("p (h d) -> p h d", h=HPC),
            )

    # ---------------- stages 2-6: attention + out-proj ----------------
    with (
        tc.tile_pool(name="sT", bufs=2, space="PSUM") as sTp,
        tc.tile_pool(name="outT", bufs=2, space="PSUM") as oTp,
        tc.tile_pool(name="ypsum", bufs=2, space="PSUM") as yTp,
        tc.tile_pool(name="pT", bufs=8) as pTp,
        tc.tile_pool(name="small", bufs=8) as smallp,
        tc.tile_pool(name="ystage", bufs=4) as ysp,
    ):
        def emit_stage6(sqb):
            # y^T[e, sqb] = Wp.T @ att^T[:, sqb]; evacuate on GPSIMD so the
            # DVE stays free for masks/normalize.
            for et in range(C // 128):
                yps_t = yTp.tile([128, QB], F32, tag="yT", name="yps")
                for kc in range(2):
                    nc.tensor.matmul(
                        yps_t[:, :],
                        lhsT=wp_sb[kc][:, et * 128:(et + 1) * 128],
                        rhs=attT_sb[kc][:, sqb * QB:(sqb + 1) * QB],
                        start=(kc == 0),
                        stop=(kc == 1),
                    )
                ys = ysp.tile([128, QB], F32, tag="ys", name="ys")
                nc.any.tensor_copy(ys[:, :], yps_t[:, :])
                nc.sync.dma_start(
                    out=yT[et * 128:(et + 1) * 128, sqb * QB:(sqb + 1) * QB],
                    in_=ys[:, :],
                )

        prev_qb = None
        for qb in (3, 2, 1, 0):
            for hp in range(2):
                ktile, qtile = kT_sb[hp], qT_sb[hp]
                ngr = 2 * (qb + 1)   # groups of 2 j-tiles each
                njt = 4 * (qb + 1)
                oT = {m2: oTp.tile([128, QB], F32, tag="oT", name="oT") for m2 in range(2)}

                def emit_omms(m2, grp, pT):
                    h = hp * 2 + m2
                    for m in range(2):
                        jt = grp * 2 + m
                        c0 = 128 * (jt - 4 * qb) if grp >= ngr - 2 else 0
                        nc.tensor.matmul(
                            oT[m2][0:D + 1, c0:QB],
                            lhsT=v_sb[jt][:, h, :],
                            rhs=pT[:, m, c0:QB],
                            start=(jt == 0),
                            stop=(jt == njt - 1),
                        )

                pend = []
                for grp in range(ngr):
                    diag = grp >= ngr - 2
                    for m2 in range(2):
                        po = m2 * D
                        # the two heads' S-matmuls contract 64 partitions
                        # each on disjoint ranges -> they run on the two
                        # 64-row PE tiles concurrently when adjacent.
                        sT = sTp.tile([128, 2, QB], F32, tag="sT", name="sT")
                        for m in range(2):
                            jt = grp * 2 + m
                            c0 = 128 * (jt - 4 * qb) if diag else 0
                            nc.tensor.matmul(
                                sT[:, m, c0:QB],
                                lhsT=ktile[po:po + D, jt * 128:(jt + 1) * 128],
                                rhs=qtile[po:po + D, qb * QB + c0:(qb + 1) * QB],
                                start=True,
                                stop=True,
                            )
                        if diag:  # causal mask on the 128-wide diagonal blocks
                            for m in range(2):
                                jt = grp * 2 + m
                                c0 = 128 * (jt - 4 * qb)
                                nc.vector.tensor_add(
                                    sT[:, m, c0:c0 + 128], sT[:, m, c0:c0 + 128], mask_sb[:, :]
                                )
                        pT = pTp.tile([128, 2, QB], F32R, tag="pT", name="pT")
                        if grp == ngr - 1:
                            for m in range(2):
                                c0 = 128 * (grp * 2 + m - 4 * qb)
                                nc.scalar.activation(pT[:, m, c0:QB], sT[:, m, c0:QB], AF.Exp)
                        else:
                            # full-tile exp: on grp == ngr-2 the columns left
                            # of c0 are stale psum, but no consumer reads
                            # pT there, so one big ACT beats two sliced ones.
                            nc.scalar.activation(pT[:, :, :], sT[:, :, :], AF.Exp)
                        pend.append((m2, grp, pT))
                        if len(pend) > 4:
                            emit_omms(*pend.pop(0))
                for ent in pend:
                    emit_omms(*ent)
                # normalize: att^T = outT[0:D] * (1/Z), Z = outT[D]
                for m2 in range(2):
                    po = m2 * D
                    zrow = smallp.tile([1, QB], F32, tag="zrow", name="zrow")
                    nc.vector.tensor_copy(zrow[:, :], oT[m2][D:D + 1, :])
                    rz = smallp.tile([1, QB], F32, tag="rz", name="rz")
                    nc.vector.reciprocal_approx_fast(out=rz[:, :], in_=zrow[:, :])
                    zs = smallp.tile([D, QB], F32, tag="zs", name="zs")
                    nc.gpsimd.partition_broadcast(zs[:, :], rz[:, :], channels=D)
                    nc.vector.tensor_mul(
                        attT_sb[hp][po:po + D, qb * QB:(qb + 1) * QB],
                        oT[m2][0:D, :],
                        zs[:, :],
                    )
                # slide the PREVIOUS q-block's out-proj in after the first
                # head pair so its y-matmuls queue behind independent work.
                if hp == 0 and prev_qb is not None:
                    emit_stage6(prev_qb)
            prev_qb = qb
        emit_stage6(prev_qb)


def build_nc():
    from contextlib import ExitStack

    nc = bacc.Bacc("TRN2", target_bir_lowering=False)
    xT = nc.dram_tensor("xT", [C, T], BF16, kind="ExternalInput")
    wqk = nc.dram_tensor("wqk", [C, 2 * CS], BF16, kind="ExternalInput")
    wv = nc.dram_tensor("wv", [C, CS], BF16, kind="ExternalInput")
    bqk = nc.dram_tensor("bqk", [2 * CS, 1], F32, kind="ExternalInput")
    wp = nc.dram_tensor("wp", [CS, C], BF16, kind="ExternalInput")
    masks = nc.dram_tensor("masks", [128, 128], F32, kind="ExternalInput")
    yT = nc.dram_tensor("yT", [C, T], F32, kind="ExternalOutput")
    with tile.TileContext(nc) as tc:
        with nc.allow_low_precision(reason="bf16 inputs / fp32r matmul; accumulation stays fp32 in PSUM"):
            with ExitStack() as ctx:
                _build_body(nc, tc, ctx, xT, wqk, wv, bqk, wp, masks, yT)
    nc.compile()
    return nc


def make_masks():
    r = np.arange(128)[:, None]
    c = np.arange(128)[None, :]
    return np.where(r <= c, np.float32(0.0), np.float32(NEG)).astype(np.float32)


def make_in_maps(x, W_qkv, b_qkv, W_proj):
    scale = np.float32(1.0 / np.sqrt(D))
    mask_h = make_masks()
    bf = ml_dtypes.bfloat16
    in_maps = []
    for i in range(NCORES):
        b, g = divmod(i, HPC)
        cs0 = g * CS
        wq = W_qkv[:, cs0:cs0 + CS] * scale
        wk = W_qkv[:, C + cs0:C + cs0 + CS]
        bq = b_qkv[cs0:cs0 + CS] * scale
        bk = b_qkv[C + cs0:C + cs0 + CS]
        in_maps.append({
            "xT": np.ascontiguousarray(x[b].T).astype(bf),
            "wqk": np.concatenate([wq, wk], axis=1).astype(bf),
            "wv": np.ascontiguousarray(W_qkv[:, 2 * C + cs0:2 * C + cs0 + CS]).astype(bf),
            "bqk": np.concatenate([bq, bk])[:, None].astype(np.float32),
            "wp": np.ascontiguousarray(W_proj[cs0:cs0 + CS, :]).astype(bf),
            "masks": mask_h,
        })
    return in_maps


_NC_CACHE = None


def _get_nc():
    global _NC_CACHE
    if _NC_CACHE is None:
        _NC_CACHE = build_nc()
    return _NC_CACHE


def gather(results, b_qkv, W_proj, b_proj):
    Y = np.zeros((B, T, C), np.float32)
    for i in range(NCORES):
        Y[i // HPC] += results[i]["yT"].T
    Y += (b_qkv[2 * C:].astype(np.float32) @ W_proj.astype(np.float32)
          + b_proj.astype(np.float32))[None, None, :]
    return Y


def kernel(x, W_qkv, b_qkv, W_proj, b_proj):
    global LAST_RESULT
    x = np.asarray(x, np.float32)
    W_qkv = np.asarray(W_qkv, np.float32)
    b_qkv = np.asarray(b_qkv, np.float32)
    W_proj = np.asarray(W_proj, np.float32)
    b_proj = np.asarray(b_proj, np.float32)

    nc = _get_nc()
    in_maps = make_in_maps(x, W_qkv, b_qkv, W_proj)
    res = run_bass_kernel_spmd(nc, in_maps, list(range(NCORES)), trace=TRACE)
    LAST_RESULT = res
    if TRACE and res.exec_time_ns is not None:
        print(f"HW exec time: {res.exec_time_ns} ns")
    return gather(res.results, b_qkv, W_proj, b_proj)


# revision 6
# speedup vs baseline: 1.0836x; 1.0836x over previous
"""Causal multi-head attention (B=2, T=2048, C=1024, H=16, d=64) on 8 trn2 cores.

Sharding: core i -> (batch b = i//4, head group g = i%4, 4 heads/core).
Data parallel over B, tensor parallel over heads; the out-proj partial sums
(contraction over this core's 256 channels) are reduced on the host during
the gather step, along with b_proj and the analytically-folded V bias.

Device kernel works entirely in [feature, token] (transposed) layout so no
on-device transposes are needed.

Perf design is driven by the PE HAM clock gate (2.4 GHz only under sustained
busy; any recurring idle re-throttles to 1.2 GHz): the whole kernel is ONE
software pipeline in which projection work ("tranches") and the out-proj are
fed to the PE as filler inside the attention units, so the PE never starves
while ACT (exp, the second-busiest engine) chases it:

  tranche(pair, tc4): Q^T,K^T 512-token block for one head pair (+V t-tiles
      on pair 0), k-loop paced by the bf16 x/w DMA stream.
  unit(pair, qb):  per j-tile g-cycles over the two heads of the pair:
      S^T (the two heads' K=64 matmuls land on the two 64-row PE tiles and
      run concurrently), causal mask (DVE), exp (ACT), PV accumulation
      [V_h | 1] lagged 2 g-cycles (row 64 = softmax Z), then att^T =
      outT[0:64] * (1/Z) -> bf16 via reciprocal_approx_fast + GPSIMD
      partition_broadcast.
  Units run in ASCENDING qb order so unit (pair, qb) only needs tranches
      0..qb -- this is what lets tranche/attention interleave at all.
  stage6(qb): y^T = Wp.T @ att^T (bf16), interleaved as filler too;
      host sums partials + transposes.

x / W_qkv / W_v / W_proj travel as bf16 (halves DMA; ~1e-3 rel err), scores
and P stay fp32(r) end-to-end, accumulation always fp32 in PSUM.
"""

import numpy as np
import ml_dtypes

import concourse.bass as bass
import concourse.mybir as mybir
from concourse import bacc
import concourse.tile as tile
from concourse.bass_utils import run_bass_kernel_spmd

B, T, C, H, D = 2, 2048, 1024, 16, 64
NCORES = 8
HPC = 4            # heads per core
CS = HPC * D       # 256 channels per core (per Q/K/V block)
KT = C // 128      # 8 contraction tiles for the projections
NT = T // 128      # 16 token tiles of 128
QB = 512           # query block (psum bank width in fp32)
NQB = T // QB      # 4 query blocks
NEG = -1e9

F32 = mybir.dt.float32
F32R = mybir.dt.float32r
BF16 = mybir.dt.bfloat16

TRACE = False
LAST_RESULT = None


def _build_body(nc, tc, ctx, xT, wqk, wv, bqk, wp, masks, yT):
    AF = mybir.ActivationFunctionType

    persist = ctx.enter_context(tc.tile_pool(name="persist", bufs=1))

    xT_sb = [persist.tile([128, T], BF16, tag=f"xT{k}", name=f"xT{k}") for k in range(KT)]
    wqk_sb = [persist.tile([128, 2 * CS], BF16, tag=f"wqk{k}", name=f"wqk{k}") for k in range(KT)]
    wv_sb = [persist.tile([128, CS], BF16, tag=f"wv{k}", name=f"wv{k}") for k in range(KT)]
    bqk_sb = [persist.tile([128, 1], F32, tag=f"bqk{c}", name=f"bqk{c}") for c in range(4)]
    wp_sb = [persist.tile([128, C], BF16, tag=f"wp{k}", name=f"wp{k}") for k in range(2)]
    mask_sb = persist.tile([128, 128], F32, tag="mask", name="mask_sb")
    qT_sb = [persist.tile([128, T], F32R, tag=f"qT{i}", name=f"qT{i}") for i in range(2)]
    kT_sb = [persist.tile([128, T], F32R, tag=f"kT{i}", name=f"kT{i}") for i in range(2)]
    v_sb = [persist.tile([128, HPC, D + 1], F32R, tag=f"v{t}", name=f"v{t}") for t in range(NT)]
    attT_sb = [persist.tile([128, T], BF16, tag=f"attT{i}", name=f"attT{i}") for i in range(2)]

    s1p = ctx.enter_context(tc.tile_pool(name="s1psum", bufs=2, space="PSUM"))
    sTp = ctx.enter_context(tc.tile_pool(name="sT", bufs=2, space="PSUM"))
    oTp = ctx.enter_context(tc.tile_pool(name="outT", bufs=2, space="PSUM"))
    yTp = ctx.enter_context(tc.tile_pool(name="ypsum", bufs=2, space="PSUM"))
    pTp = ctx.enter_context(tc.tile_pool(name="pT", bufs=6))
    smallp = ctx.enter_context(tc.tile_pool(name="small", bufs=8))
    ysp = ctx.enter_context(tc.tile_pool(name="ystage", bufs=4))

    # DMA order = consumption order: (wqk[k], xT[k]) pairs so the first
    # tranche's k-loop starts after ~one tile; the rest lands under compute.
    for k in range(KT):
        nc.sync.dma_start(out=wqk_sb[k][:, :], in_=wqk[k * 128:(k + 1) * 128, :])
        nc.sync.dma_start(out=xT_sb[k][:, :], in_=xT[k * 128:(k + 1) * 128, :])
    for c4 in range(4):
        nc.sync.dma_start(out=bqk_sb[c4][:, :], in_=bqk[c4 * 128:(c4 + 1) * 128, :])
    for k in range(KT):
        nc.sync.dma_start(out=wv_sb[k][:, :], in_=wv[k * 128:(k + 1) * 128, :])
    for k in range(2):
        nc.sync.dma_start(out=wp_sb[k][:, :], in_=wp[k * 128:(k + 1) * 128, :])
    nc.sync.dma_start(out=mask_sb[:, :], in_=masks[:, :])

    ones_f32 = persist.tile([128, 4], F32, tag="ones_f32", name="ones_f32")
    nc.vector.memset(ones_f32[:, :], 1.0)
    for t in range(NT):
        nc.vector.tensor_copy(v_sb[t][:, :, D], ones_f32[:, :])

    # ---------------- stage-1 tranches (filler chunks) ----------------
    def tranche_chunks(pair, tc4):
        def qk_chunk(ct, dst):
            def run():
                ps = s1p.tile([128, QB], F32, tag="ps", name="ps")
                for k in range(KT):
                    nc.tensor.matmul(
                        ps[:, :],
                        lhsT=wqk_sb[k][:, ct * 128:(ct + 1) * 128],
                        rhs=xT_sb[k][:, tc4 * QB:(tc4 + 1) * QB],
                        start=(k == 0),
                        stop=(k == KT - 1),
                    )
                nc.vector.tensor_scalar_add(
                    dst[:, tc4 * QB:(tc4 + 1) * QB], ps[:, :], bqk_sb[ct][:, :]
                )
            return run

        def v_chunk(t):
            def run():
                ps = s1p.tile([128, QB], F32, tag="ps", name="ps")
                for k in range(KT):
                    nc.tensor.matmul(
                        ps[:, 0:CS],
                        lhsT=xT_sb[k][:, t * 128:(t + 1) * 128],
                        rhs=wv_sb[k][:, :],
                        start=(k == 0),
                        stop=(k == KT - 1),
                    )
                nc.any.tensor_copy(
                    v_sb[t][:, :, 0:D],
                    ps[:, 0:CS].rearrange("p (h d) -> p h d", h=HPC),
                )
            return run

        chunks = [qk_chunk(pair, qT_sb[pair]), qk_chunk(2 + pair, kT_sb[pair])]
        if pair == 0:  # V covers both pairs' channels; compute once
            chunks += [v_chunk(t) for t in range(tc4 * 4, tc4 * 4 + 4)]
        return chunks

    # ---------------- stage 6 (filler chunks) ----------------
    def stage6_chunks(sqb):
        def et_chunk(et):
            def run():
                yps_t = yTp.tile([128, QB], F32, tag="yT", name="yps")
                for kc in range(2):
                    nc.tensor.matmul(
                        yps_t[:, :],
                        lhsT=wp_sb[kc][:, et * 128:(et + 1) * 128],
                        rhs=attT_sb[kc][:, sqb * QB:(sqb + 1) * QB],
                        start=(kc == 0),
                        stop=(kc == 1),
                    )
                ys = ysp.tile([128, QB], F32, tag="ys", name="ys")
                if et % 2 == 0:
                    nc.vector.tensor_copy(ys[:, :], yps_t[:, :])
                else:
                    nc.scalar.activation(ys[:, :], yps_t[:, :], AF.Copy)
                nc.sync.dma_start(
                    out=yT[et * 128:(et + 1) * 128, sqb * QB:(sqb + 1) * QB],
                    in_=ys[:, :],
                )
            return run
        return [et_chunk(et) for et in range(C // 128)]

    # ---------------- attention unit ----------------
    def unit(pair, qb, filler):
        ktile, qtile = kT_sb[pair], qT_sb[pair]
        njt = 4 * (qb + 1)
        oT = [oTp.tile([128, QB], F32, tag="oT", name="oT") for _ in range(2)]

        def emit_omm(m2, jt, pT):
            h = pair * 2 + m2
            c0 = 128 * (jt - 4 * qb) if jt >= njt - 4 else 0
            nc.tensor.matmul(
                oT[m2][0:D + 1, c0:QB],
                lhsT=v_sb[jt][:, h, :],
                rhs=pT[:, c0:QB],
                start=(jt == 0),
                stop=(jt == njt - 1),
            )

        pend = []
        for jt in range(njt):
            diag = jt >= njt - 4
            c0 = 128 * (jt - 4 * qb) if diag else 0
            for m2 in range(2):
                po = m2 * D
                # the two heads' S-matmuls contract 64 partitions each on
                # disjoint ranges -> they run on the two 64-row PE tiles
                # concurrently when adjacent in the queue.
                sT = sTp.tile([128, QB], F32, tag="sT", name="sT")
                nc.tensor.matmul(
                    sT[:, c0:QB],
                    lhsT=ktile[po:po + D, jt * 128:(jt + 1) * 128],
                    rhs=qtile[po:po + D, qb * QB + c0:(qb + 1) * QB],
                    start=True,
                    stop=True,
                )
                if diag:  # causal mask on the 128-wide diagonal block
                    nc.vector.tensor_add(
                        sT[:, c0:c0 + 128], sT[:, c0:c0 + 128], mask_sb[:, :]
                    )
                pT = pTp.tile([128, QB], F32R, tag="pT", name="pT")
                nc.scalar.activation(pT[:, c0:QB], sT[:, c0:QB], AF.Exp)
                pend.append((m2, jt, pT))
            while len(pend) > 4:   # PV lags 2 g-cycles behind exp
                emit_omm(*pend.pop(0))
            if filler:             # spread filler over remaining g-cycles
                n = -(-len(filler) // (njt - jt))
                for _ in range(n):
                    filler.pop(0)()
        for ent in pend:
            emit_omm(*ent)
        while filler:
            filler.pop(0)()
        # normalize: att^T = outT[0:D] * (1/Z), Z = outT[D]
        for m2 in range(2):
            po = m2 * D
            zrow = smallp.tile([1, QB], F32, tag="zrow", name="zrow")
            nc.vector.tensor_copy(zrow[:, :], oT[m2][D:D + 1, :])
            rz = smallp.tile([1, QB], F32, tag="rz", name="rz")
            nc.vector.reciprocal_approx_fast(out=rz[:, :], in_=zrow[:, :])
            zs = smallp.tile([D, QB], F32, tag="zs", name="zs")
            nc.gpsimd.partition_broadcast(zs[:, :], rz[:, :], channels=D)
            nc.vector.tensor_mul(
                attT_sb[pair][po:po + D, qb * QB:(qb + 1) * QB],
                oT[m2][0:D, :],
                zs[:, :],
            )

    # ---------------- the pipeline ----------------
    for c in tranche_chunks(0, 0):
        c()
    for tc4 in range(NQB):
        unit(0, tc4, tranche_chunks(1, tc4))
        f2 = tranche_chunks(0, tc4 + 1) if tc4 < NQB - 1 else []
        if tc4 >= 1:
            f2 = f2 + stage6_chunks(tc4 - 1)
        unit(1, tc4, f2)
    for c in stage6_chunks(NQB - 1):
        c()


def build_nc():
    from contextlib import ExitStack

    nc = bacc.Bacc("TRN2", target_bir_lowering=False)
    xT = nc.dram_tensor("xT", [C, T], BF16, kind="ExternalInput")
    wqk = nc.dram_tensor("wqk", [C, 2 * CS], BF16, kind="ExternalInput")
    wv = nc.dram_tensor("wv", [C, CS], BF16, kind="ExternalInput")
    bqk = nc.dram_tensor("bqk", [2 * CS, 1], F32, kind="ExternalInput")
    wp = nc.dram_tensor("wp", [CS, C], BF16, kind="ExternalInput")
    masks = nc.dram_tensor("masks", [128, 128], F32, kind="ExternalInput")
    yT = nc.dram_tensor("yT", [C, T], F32, kind="ExternalOutput")
    with tile.TileContext(nc) as tc:
        with nc.allow_low_precision(reason="bf16 inputs / fp32r matmul; accumulation stays fp32 in PSUM"):
            with ExitStack() as ctx:
                _build_body(nc, tc, ctx, xT, wqk, wv, bqk, wp, masks, yT)
    nc.compile()
    return nc


def make_masks():
    r = np.arange(128)[:, None]
    c = np.arange(128)[None, :]
    return np.where(r <= c, np.float32(0.0), np.float32(NEG)).astype(np.float32)


def make_in_maps(x, W_qkv, b_qkv, W_proj):
    scale = np.float32(1.0 / np.sqrt(D))
    mask_h = make_masks()
    bf = ml_dtypes.bfloat16
    in_maps = []
    for i in range(NCORES):
        b, g = divmod(i, HPC)
        cs0 = g * CS
        wq = W_qkv[:, cs0:cs0 + CS] * scale
        wk = W_qkv[:, C + cs0:C + cs0 + CS]
        bq = b_qkv[cs0:cs0 + CS] * scale
        bk = b_qkv[C + cs0:C + cs0 + CS]
        in_maps.append({
            "xT": np.ascontiguousarray(x[b].T).astype(bf),
            "wqk": np.concatenate([wq, wk], axis=1).astype(bf),
            "wv": np.ascontiguousarray(W_qkv[:, 2 * C + cs0:2 * C + cs0 + CS]).astype(bf),
            "bqk": np.concatenate([bq, bk])[:, None].astype(np.float32),
            "wp": np.ascontiguousarray(W_proj[cs0:cs0 + CS, :]).astype(bf),
            "masks": mask_h,
        })
    return in_maps


_NC_CACHE = None


def _get_nc():
    global _NC_CACHE
    if _NC_CACHE is None:
        _NC_CACHE = build_nc()
    return _NC_CACHE


def gather(results, b_qkv, W_proj, b_proj):
    Y = np.zeros((B, T, C), np.float32)
    for i in range(NCORES):
        Y[i // HPC] += results[i]["yT"].T
    Y += (b_qkv[2 * C:].astype(np.float32) @ W_proj.astype(np.float32)
          + b_proj.astype(np.float32))[None, None, :]
    return Y


def kernel(x, W_qkv, b_qkv, W_proj, b_proj):
    global LAST_RESULT
    x = np.asarray(x, np.float32)
    W_qkv = np.asarray(W_qkv, np.float32)
    b_qkv = np.asarray(b_qkv, np.float32)
    W_proj = np.asarray(W_proj, np.float32)
    b_proj = np.asarray(b_proj, np.float32)

    nc = _get_nc()
    in_maps = make_in_maps(x, W_qkv, b_qkv, W_proj)
    res = run_bass_kernel_spmd(nc, in_maps, list(range(NCORES)), trace=TRACE)
    LAST_RESULT = res
    if TRACE and res.exec_time_ns is not None:
        print(f"HW exec time: {res.exec_time_ns} ns")
    return gather(res.results, b_qkv, W_proj, b_proj)


# revision 8
# speedup vs baseline: 1.3441x; 1.2404x over previous
"""Causal multi-head attention (B=2, T=2048, C=1024, H=16, d=64) on 8 trn2 cores.

Sharding: core i -> (batch b = i//4, head group g = i%4, 4 heads/core).
Data parallel over B, tensor parallel over heads; the out-proj partial sums
(contraction over this core's 256 channels) are reduced on the host during
the gather step, along with b_proj and the analytically-folded V bias.

Device kernel works entirely in [feature, token] (transposed) layout so no
on-device transposes are needed.

Perf design is driven by the PE HAM clock gate (2.4 GHz only under sustained
busy; recurring idle re-throttles to 1.2 GHz): the whole kernel is ONE
software pipeline in which projection work and the out-proj are fed to the
PE as filler inside the attention units, so the PE never starves while ACT
(exp, the second-busiest engine) chases it:

  QK(pair, tc4):  Q^T,K^T 512-token block for one head pair, k-loop paced
      by the bf16 x/w DMA stream at the front of the kernel.
  V(tc4):         4 V t-tiles (natural layout, stage-4 lhsT, both pairs'
      channels at once), ones column appended (row 64 = softmax Z).
  unit(pair, qb): attention g-cycles of 2 j-tiles x 2 heads: S^T (the two
      heads' K=64 matmuls land on the two 64-row PE tiles and run
      CONCURRENTLY when adjacent), causal mask (DVE), one exp per
      head-group on ACT (sliced so stale-psum cols are never consumed),
      PV accumulation lagged 2 g-cycles, then att^T = outT[0:64] * (1/Z)
      -> bf16 via reciprocal_approx_fast + GPSIMD partition_broadcast.
  Units run in ASCENDING qb order so unit (pair, qb) only needs q/k/v
      blocks 0..qb -- this is what lets projection/attention interleave.
  S6(qb): y^T = Wp.T @ att^T (bf16) as late filler; host sums partials.

x / W_qkv / W_v / W_proj travel as bf16 (halves DMA; ~1e-3 rel err), scores
and P stay fp32(r) end-to-end, accumulation always fp32 in PSUM.
"""

import numpy as np
import ml_dtypes

import concourse.bass as bass
import concourse.mybir as mybir
from concourse import bacc
import concourse.tile as tile
from concourse.bass_utils import run_bass_kernel_spmd

B, T, C, H, D = 2, 2048, 1024, 16, 64
NCORES = 8
HPC = 4            # heads per core
CS = HPC * D       # 256 channels per core (per Q/K/V block)
KT = C // 128      # 8 contraction tiles for the projections
NT = T // 128      # 16 token tiles of 128
QB = 512           # query block (psum bank width in fp32)
NQB = T // QB      # 4 query blocks
NEG = -1e9

F32 = mybir.dt.float32
F32R = mybir.dt.float32r
BF16 = mybir.dt.bfloat16

TRACE = False
LAST_RESULT = None


def _build_body(nc, tc, ctx, xT, wqk, wv, bqk, wp, masks, yT):
    AF = mybir.ActivationFunctionType

    persist = ctx.enter_context(tc.tile_pool(name="persist", bufs=1))

    xT_sb = [persist.tile([128, T], BF16, tag=f"xT{k}", name=f"xT{k}") for k in range(KT)]
    wqk_sb = [persist.tile([128, 2 * CS], BF16, tag=f"wqk{k}", name=f"wqk{k}") for k in range(KT)]
    wv_sb = [persist.tile([128, CS], BF16, tag=f"wv{k}", name=f"wv{k}") for k in range(KT)]
    bqk_sb = [persist.tile([128, 1], F32, tag=f"bqk{c}", name=f"bqk{c}") for c in range(4)]
    wp_sb = [persist.tile([128, C], BF16, tag=f"wp{k}", name=f"wp{k}") for k in range(2)]
    mask_sb = persist.tile([128, 128], F32, tag="mask", name="mask_sb")
    qT_sb = [persist.tile([128, T], F32R, tag=f"qT{i}", name=f"qT{i}") for i in range(2)]
    kT_sb = [persist.tile([128, T], F32R, tag=f"kT{i}", name=f"kT{i}") for i in range(2)]
    v_sb = [persist.tile([128, HPC, D + 1], F32R, tag=f"v{t}", name=f"v{t}") for t in range(NT)]
    attT_sb = [persist.tile([128, T], BF16, tag=f"attT{i}", name=f"attT{i}") for i in range(2)]

    # PSUM: sT 2x2 banks + oT 2 + fill (shared stage1/stage6) 2 = 8 banks
    sTp = ctx.enter_context(tc.tile_pool(name="sT", bufs=2, space="PSUM"))
    oTp = ctx.enter_context(tc.tile_pool(name="outT", bufs=2, space="PSUM"))
    fillp = ctx.enter_context(tc.tile_pool(name="fillp", bufs=2, space="PSUM"))
    pTp = ctx.enter_context(tc.tile_pool(name="pT", bufs=6))
    smallp = ctx.enter_context(tc.tile_pool(name="small", bufs=8))
    ysp = ctx.enter_context(tc.tile_pool(name="ystage", bufs=4))

    # DMA order = consumption order: (wqk[k], xT[k]) pairs so the first
    # QK chunk's k-loop starts after ~one tile; the rest lands under compute.
    for k in range(KT):
        nc.sync.dma_start(out=wqk_sb[k][:, :], in_=wqk[k * 128:(k + 1) * 128, :])
        nc.sync.dma_start(out=xT_sb[k][:, :], in_=xT[k * 128:(k + 1) * 128, :])
    for c4 in range(4):
        nc.sync.dma_start(out=bqk_sb[c4][:, :], in_=bqk[c4 * 128:(c4 + 1) * 128, :])
    for k in range(KT):
        nc.sync.dma_start(out=wv_sb[k][:, :], in_=wv[k * 128:(k + 1) * 128, :])
    for k in range(2):
        nc.sync.dma_start(out=wp_sb[k][:, :], in_=wp[k * 128:(k + 1) * 128, :])
    nc.sync.dma_start(out=mask_sb[:, :], in_=masks[:, :])

    ones_f32 = persist.tile([128, 4], F32, tag="ones_f32", name="ones_f32")
    nc.vector.memset(ones_f32[:, :], 1.0)
    for t in range(NT):
        nc.vector.tensor_copy(v_sb[t][:, :, D], ones_f32[:, :])

    # ---------------- projection chunks (filler) ----------------
    def qk_chunks(pair, tc4):
        def qk_chunk(ct, dst):
            def run():
                ps = fillp.tile([128, QB], F32, tag="fp", name="fp")
                for k in range(KT):
                    nc.tensor.matmul(
                        ps[:, :],
                        lhsT=wqk_sb[k][:, ct * 128:(ct + 1) * 128],
                        rhs=xT_sb[k][:, tc4 * QB:(tc4 + 1) * QB],
                        start=(k == 0),
                        stop=(k == KT - 1),
                    )
                nc.vector.tensor_scalar_add(
                    dst[:, tc4 * QB:(tc4 + 1) * QB], ps[:, :], bqk_sb[ct][:, :]
                )
            return run
        return [qk_chunk(pair, qT_sb[pair]), qk_chunk(2 + pair, kT_sb[pair])]

    def v_chunks(tc4):
        def v_chunk(t):
            def run():
                ps = fillp.tile([128, QB], F32, tag="fp", name="fp")
                for k in range(KT):
                    nc.tensor.matmul(
                        ps[:, 0:CS],
                        lhsT=xT_sb[k][:, t * 128:(t + 1) * 128],
                        rhs=wv_sb[k][:, :],
                        start=(k == 0),
                        stop=(k == KT - 1),
                    )
                nc.any.tensor_copy(
                    v_sb[t][:, :, 0:D],
                    ps[:, 0:CS].rearrange("p (h d) -> p h d", h=HPC),
                )
            return run
        return [v_chunk(t) for t in range(tc4 * 4, tc4 * 4 + 4)]

    # ---------------- stage 6 chunks (filler) ----------------
    def s6_chunks(sqb):
        def et_chunk(et):
            def run():
                yps_t = fillp.tile([128, QB], F32, tag="fp", name="fp")
                for kc in range(2):
                    nc.tensor.matmul(
                        yps_t[:, :],
                        lhsT=wp_sb[kc][:, et * 128:(et + 1) * 128],
                        rhs=attT_sb[kc][:, sqb * QB:(sqb + 1) * QB],
                        start=(kc == 0),
                        stop=(kc == 1),
                    )
                ys = ysp.tile([128, QB], F32, tag="ys", name="ys")
                nc.vector.tensor_copy(ys[:, :], yps_t[:, :])
                nc.sync.dma_start(
                    out=yT[et * 128:(et + 1) * 128, sqb * QB:(sqb + 1) * QB],
                    in_=ys[:, :],
                )
            return run
        return [et_chunk(et) for et in range(C // 128)]

    # ---------------- attention unit ----------------
    def unit(pair, qb, filler):
        ktile, qtile = kT_sb[pair], qT_sb[pair]
        ngr = 2 * (qb + 1)
        njt = 4 * (qb + 1)
        oT = [oTp.tile([128, QB], F32, tag="oT", name="oT") for _ in range(2)]

        def emit_omms(m2, grp, pT):
            h = pair * 2 + m2
            for m in range(2):
                jt = grp * 2 + m
                c0 = 128 * (jt - 4 * qb) if grp >= ngr - 2 else 0
                nc.tensor.matmul(
                    oT[m2][0:D + 1, c0:QB],
                    lhsT=v_sb[jt][:, h, :],
                    rhs=pT[:, m, c0:QB],
                    start=(jt == 0),
                    stop=(jt == njt - 1),
                )

        pend = []
        for grp in range(ngr):
            diag = grp >= ngr - 2
            for m2 in range(2):
                po = m2 * D
                # the two heads' S-matmuls contract 64 partitions each on
                # disjoint ranges -> they run on the two 64-row PE tiles
                # concurrently when adjacent in the queue.
                sT = sTp.tile([128, 2, QB], F32, tag="sT", name="sT")
                for m in range(2):
                    jt = grp * 2 + m
                    c0 = 128 * (jt - 4 * qb) if diag else 0
                    nc.tensor.matmul(
                        sT[:, m, c0:QB],
                        lhsT=ktile[po:po + D, jt * 128:(jt + 1) * 128],
                        rhs=qtile[po:po + D, qb * QB + c0:(qb + 1) * QB],
                        start=True,
                        stop=True,
                    )
                if diag:  # causal mask on the two 128-wide diagonal blocks
                    for m in range(2):
                        c0 = 128 * (grp * 2 + m - 4 * qb)
                        nc.vector.tensor_add(
                            sT[:, m, c0:c0 + 128], sT[:, m, c0:c0 + 128], mask_sb[:, :]
                        )
                pT = pTp.tile([128, 2, QB], F32R, tag="pT", name="pT")
                # one exp per head-group; on diag groups slice from m0's
                # first valid column -- the m1 columns [c0, c0+128) this
                # covers are stale psum that no PV matmul ever reads.
                ce = 128 * (grp * 2 - 4 * qb) if diag else 0
                nc.scalar.activation(pT[:, :, ce:QB], sT[:, :, ce:QB], AF.Exp)
                pend.append((m2, grp, pT))
            while len(pend) > 4:   # PV lags 2 g-cycles behind exp
                emit_omms(*pend.pop(0))
            if filler:             # spread filler over remaining g-cycles
                n = -(-len(filler) // (ngr - grp))
                for _ in range(n):
                    filler.pop(0)()
        for ent in pend:
            emit_omms(*ent)
        while filler:
            filler.pop(0)()
        # normalize: att^T = outT[0:D] * (1/Z), Z = outT[D]
        for m2 in range(2):
            po = m2 * D
            zrow = smallp.tile([1, QB], F32, tag="zrow", name="zrow")
            nc.vector.tensor_copy(zrow[:, :], oT[m2][D:D + 1, :])
            rz = smallp.tile([1, QB], F32, tag="rz", name="rz")
            nc.vector.reciprocal_approx_fast(out=rz[:, :], in_=zrow[:, :])
            zs = smallp.tile([D, QB], F32, tag="zs", name="zs")
            nc.gpsimd.partition_broadcast(zs[:, :], rz[:, :], channels=D)
            nc.vector.tensor_mul(
                attT_sb[pair][po:po + D, qb * QB:(qb + 1) * QB],
                oT[m2][0:D, :],
                zs[:, :],
            )

    # ---------------- the pipeline ----------------
    # fillers are scheduled as late as dependencies allow, to keep the PE
    # fed (warm) during the ACT-heavy later units.
    for c in qk_chunks(0, 0) + v_chunks(0):
        c()
    unit(0, 0, qk_chunks(1, 0))
    unit(1, 0, qk_chunks(0, 1))
    unit(0, 1, qk_chunks(1, 1) + v_chunks(1))
    unit(1, 1, qk_chunks(0, 2) + s6_chunks(0))
    unit(0, 2, qk_chunks(1, 2) + v_chunks(2))
    unit(1, 2, qk_chunks(0, 3) + v_chunks(3))
    unit(0, 3, qk_chunks(1, 3) + s6_chunks(1))
    unit(1, 3, s6_chunks(2))
    for c in s6_chunks(3):
        c()


def build_nc():
    from contextlib import ExitStack

    nc = bacc.Bacc("TRN2", target_bir_lowering=False)
    xT = nc.dram_tensor("xT", [C, T], BF16, kind="ExternalInput")
    wqk = nc.dram_tensor("wqk", [C, 2 * CS], BF16, kind="ExternalInput")
    wv = nc.dram_tensor("wv", [C, CS], BF16, kind="ExternalInput")
    bqk = nc.dram_tensor("bqk", [2 * CS, 1], F32, kind="ExternalInput")
    wp = nc.dram_tensor("wp", [CS, C], BF16, kind="ExternalInput")
    masks = nc.dram_tensor("masks", [128, 128], F32, kind="ExternalInput")
    yT = nc.dram_tensor("yT", [C, T], F32, kind="ExternalOutput")
    with tile.TileContext(nc) as tc:
        with nc.allow_low_precision(reason="bf16 inputs / fp32r matmul; accumulation stays fp32 in PSUM"):
            with ExitStack() as ctx:
                _build_body(nc, tc, ctx, xT, wqk, wv, bqk, wp, masks, yT)
    nc.compile()
    return nc


def make_masks():
    r = np.arange(128)[:, None]
    c = np.arange(128)[None, :]
    return np.where(r <= c, np.float32(0.0), np.float32(NEG)).astype(np.float32)


def make_in_maps(x, W_qkv, b_qkv, W_proj):
    scale = np.float32(1.0 / np.sqrt(D))
    mask_h = make_masks()
    bf = ml_dtypes.bfloat16
    in_maps = []
    for i in range(NCORES):
        b, g = divmod(i, HPC)
        cs0 = g * CS
        wq = W_qkv[:, cs0:cs0 + CS] * scale
        wk = W_qkv[:, C + cs0:C + cs0 + CS]
        bq = b_qkv[cs0:cs0 + CS] * scale
        bk = b_qkv[C + cs0:C + cs0 + CS]
        in_maps.append({
            "xT": np.ascontiguousarray(x[b].T).astype(bf),
            "wqk": np.concatenate([wq, wk], axis=1).astype(bf),
            "wv": np.ascontiguousarray(W_qkv[:, 2 * C + cs0:2 * C + cs0 + CS]).astype(bf),
            "bqk": np.concatenate([bq, bk])[:, None].astype(np.float32),
            "wp": np.ascontiguousarray(W_proj[cs0:cs0 + CS, :]).astype(bf),
            "masks": mask_h,
        })
    return in_maps


_NC_CACHE = None


def _get_nc():
    global _NC_CACHE
    if _NC_CACHE is None:
        _NC_CACHE = build_nc()
    return _NC_CACHE


def gather(results, b_qkv, W_proj, b_proj):
    Y = np.zeros((B, T, C), np.float32)
    for i in range(NCORES):
        Y[i // HPC] += results[i]["yT"].T
    Y += (b_qkv[2 * C:].astype(np.float32) @ W_proj.astype(np.float32)
          + b_proj.astype(np.float32))[None, None, :]
    return Y


def kernel(x, W_qkv, b_qkv, W_proj, b_proj):
    global LAST_RESULT
    x = np.asarray(x, np.float32)
    W_qkv = np.asarray(W_qkv, np.float32)
    b_qkv = np.asarray(b_qkv, np.float32)
    W_proj = np.asarray(W_proj, np.float32)
    b_proj = np.asarray(b_proj, np.float32)

    nc = _get_nc()
    in_maps = make_in_maps(x, W_qkv, b_qkv, W_proj)
    res = run_bass_kernel_spmd(nc, in_maps, list(range(NCORES)), trace=TRACE)
    LAST_RESULT = res
    if TRACE and res.exec_time_ns is not None:
        print(f"HW exec time: {res.exec_time_ns} ns")
    return gather(res.results, b_qkv, W_proj, b_proj)


# revision 10
# speedup vs baseline: 1.3589x; 1.0110x over previous
"""Causal multi-head attention (B=2, T=2048, C=1024, H=16, d=64) on 8 trn2 cores.

Sharding: core i -> (batch b = i//4, head group g = i%4, 4 heads/core).
Data parallel over B, tensor parallel over heads; the out-proj partial sums
(contraction over this core's 256 channels) are reduced on the host during
the gather step, along with b_proj and the analytically-folded V bias.

Device kernel works entirely in [feature, token] (transposed) layout so no
on-device transposes are needed.

Perf design is driven by the PE HAM clock gate (2.4 GHz only under sustained
busy; recurring idle re-throttles to 1.2 GHz): the whole kernel is ONE
software pipeline in which projection work and the out-proj are fed to the
PE as filler inside the attention units, so the PE never starves while ACT
(exp, the second-busiest engine) chases it:

  QK(pair, tc4):  Q^T,K^T 512-token block for one head pair, k-loop paced
      by the bf16 x/w DMA stream at the front of the kernel.
  V(tc4):         4 V t-tiles (natural layout, stage-4 lhsT, both pairs'
      channels at once), ones column appended (row 64 = softmax Z).
  unit(pair, qb): attention g-cycles of 2 j-tiles x 2 heads: S^T (the two
      heads' K=64 matmuls land on the two 64-row PE tiles and run
      CONCURRENTLY when adjacent), causal mask (DVE), one exp per
      head-group on ACT (sliced so stale-psum cols are never consumed),
      PV accumulation lagged 2 g-cycles, then att^T = outT[0:64] * (1/Z)
      -> bf16 via reciprocal_approx_fast + GPSIMD partition_broadcast.
  Units run in ASCENDING qb order so unit (pair, qb) only needs q/k/v
      blocks 0..qb -- this is what lets projection/attention interleave.
  S6(qb): y^T = Wp.T @ att^T (bf16) as late filler; host sums partials.

x / W_qkv / W_v / W_proj travel as bf16 (halves DMA; ~1e-3 rel err), scores
and P stay fp32(r) end-to-end, accumulation always fp32 in PSUM.
"""

import numpy as np
import ml_dtypes

import concourse.bass as bass
import concourse.mybir as mybir
from concourse import bacc
import concourse.tile as tile
from concourse.bass_utils import run_bass_kernel_spmd

B, T, C, H, D = 2, 2048, 1024, 16, 64
NCORES = 8
HPC = 4            # heads per core
CS = HPC * D       # 256 channels per core (per Q/K/V block)
KT = C // 128      # 8 contraction tiles for the projections
NT = T // 128      # 16 token tiles of 128
QB = 512           # query block (psum bank width in fp32)
NQB = T // QB      # 4 query blocks
NEG = -1e9

F32 = mybir.dt.float32
F32R = mybir.dt.float32r
BF16 = mybir.dt.bfloat16

TRACE = False
LAST_RESULT = None


def _build_body(nc, tc, ctx, xT, wqk, wv, bqk, wp, masks, yT):
    AF = mybir.ActivationFunctionType

    persist = ctx.enter_context(tc.tile_pool(name="persist", bufs=1))

    xT_sb = [persist.tile([128, T], BF16, tag=f"xT{k}", name=f"xT{k}") for k in range(KT)]
    wqk_sb = [persist.tile([128, 2 * CS], BF16, tag=f"wqk{k}", name=f"wqk{k}") for k in range(KT)]
    wv_sb = [persist.tile([128, CS], BF16, tag=f"wv{k}", name=f"wv{k}") for k in range(KT)]
    bqk_sb = [persist.tile([128, 1], F32, tag=f"bqk{c}", name=f"bqk{c}") for c in range(4)]
    wp_sb = [persist.tile([128, C], BF16, tag=f"wp{k}", name=f"wp{k}") for k in range(2)]
    mask_sb = persist.tile([128, 128], F32, tag="mask", name="mask_sb")
    qT_sb = [persist.tile([128, T], F32R, tag=f"qT{i}", name=f"qT{i}") for i in range(2)]
    kT_sb = [persist.tile([128, T], F32R, tag=f"kT{i}", name=f"kT{i}") for i in range(2)]
    v_sb = [persist.tile([128, HPC, D + 1], F32R, tag=f"v{t}", name=f"v{t}") for t in range(NT)]
    attT_sb = [persist.tile([128, T], BF16, tag=f"attT{i}", name=f"attT{i}") for i in range(2)]

    # PSUM: sT 2x2 banks + oT 2 + fill (shared stage1/stage6) 2 = 8 banks
    sTp = ctx.enter_context(tc.tile_pool(name="sT", bufs=2, space="PSUM"))
    oTp = ctx.enter_context(tc.tile_pool(name="outT", bufs=2, space="PSUM"))
    fillp = ctx.enter_context(tc.tile_pool(name="fillp", bufs=2, space="PSUM"))
    pTp = ctx.enter_context(tc.tile_pool(name="pT", bufs=6))
    smallp = ctx.enter_context(tc.tile_pool(name="small", bufs=8))
    ysp = ctx.enter_context(tc.tile_pool(name="ystage", bufs=4))

    # DMA order = consumption order; x lands in token-quarter slices so the
    # first QK/V chunks (and with them the whole pipeline) start after ~2MB
    # instead of waiting for the full input.
    def dma_x(tc4):
        for k in range(KT):
            nc.sync.dma_start(
                out=xT_sb[k][:, tc4 * QB:(tc4 + 1) * QB],
                in_=xT[k * 128:(k + 1) * 128, tc4 * QB:(tc4 + 1) * QB],
            )
    for k in range(KT):
        nc.sync.dma_start(out=wqk_sb[k][:, :], in_=wqk[k * 128:(k + 1) * 128, :])
    dma_x(0)
    for k in range(KT):
        nc.sync.dma_start(out=wv_sb[k][:, :], in_=wv[k * 128:(k + 1) * 128, :])
    for c4 in range(4):
        nc.sync.dma_start(out=bqk_sb[c4][:, :], in_=bqk[c4 * 128:(c4 + 1) * 128, :])
    dma_x(1)
    for k in range(2):
        nc.sync.dma_start(out=wp_sb[k][:, :], in_=wp[k * 128:(k + 1) * 128, :])
    nc.sync.dma_start(out=mask_sb[:, :], in_=masks[:, :])
    dma_x(2)
    dma_x(3)

    ones_f32 = persist.tile([128, 4], F32, tag="ones_f32", name="ones_f32")
    nc.vector.memset(ones_f32[:, :], 1.0)
    for t in range(NT):
        nc.vector.tensor_copy(v_sb[t][:, :, D], ones_f32[:, :])

    # ---------------- projection chunks (filler) ----------------
    def qk_chunks(pair, tc4):
        def qk_chunk(ct, dst):
            def run():
                ps = fillp.tile([128, QB], F32, tag="fp", name="fp")
                for k in range(KT):
                    nc.tensor.matmul(
                        ps[:, :],
                        lhsT=wqk_sb[k][:, ct * 128:(ct + 1) * 128],
                        rhs=xT_sb[k][:, tc4 * QB:(tc4 + 1) * QB],
                        start=(k == 0),
                        stop=(k == KT - 1),
                    )
                nc.vector.tensor_scalar_add(
                    dst[:, tc4 * QB:(tc4 + 1) * QB], ps[:, :], bqk_sb[ct][:, :]
                )
            return run
        return [qk_chunk(pair, qT_sb[pair]), qk_chunk(2 + pair, kT_sb[pair])]

    def v_chunks(tc4):
        def v_chunk(t):
            def run():
                ps = fillp.tile([128, QB], F32, tag="fp", name="fp")
                for k in range(KT):
                    nc.tensor.matmul(
                        ps[:, 0:CS],
                        lhsT=xT_sb[k][:, t * 128:(t + 1) * 128],
                        rhs=wv_sb[k][:, :],
                        start=(k == 0),
                        stop=(k == KT - 1),
                    )
                nc.any.tensor_copy(
                    v_sb[t][:, :, 0:D],
                    ps[:, 0:CS].rearrange("p (h d) -> p h d", h=HPC),
                )
            return run
        return [v_chunk(t) for t in range(tc4 * 4, tc4 * 4 + 4)]

    # ---------------- stage 6 chunks (filler) ----------------
    def s6_chunks(sqb):
        def et_chunk(et):
            def run():
                yps_t = fillp.tile([128, QB], F32, tag="fp", name="fp")
                for kc in range(2):
                    nc.tensor.matmul(
                        yps_t[:, :],
                        lhsT=wp_sb[kc][:, et * 128:(et + 1) * 128],
                        rhs=attT_sb[kc][:, sqb * QB:(sqb + 1) * QB],
                        start=(kc == 0),
                        stop=(kc == 1),
                    )
                ys = ysp.tile([128, QB], BF16, tag="ys", name="ys")
                nc.vector.tensor_copy(ys[:, :], yps_t[:, :])
                nc.sync.dma_start(
                    out=yT[et * 128:(et + 1) * 128, sqb * QB:(sqb + 1) * QB],
                    in_=ys[:, :],
                )
            return run
        return [et_chunk(et) for et in range(C // 128)]

    # ---------------- attention unit ----------------
    def unit(pair, qb, filler):
        ktile, qtile = kT_sb[pair], qT_sb[pair]
        ngr = 2 * (qb + 1)
        njt = 4 * (qb + 1)
        oT = [oTp.tile([128, QB], F32, tag="oT", name="oT") for _ in range(2)]

        def emit_omms(m2, grp, pT):
            h = pair * 2 + m2
            for m in range(2):
                jt = grp * 2 + m
                c0 = 128 * (jt - 4 * qb) if grp >= ngr - 2 else 0
                nc.tensor.matmul(
                    oT[m2][0:D + 1, c0:QB],
                    lhsT=v_sb[jt][:, h, :],
                    rhs=pT[:, m, c0:QB],
                    start=(jt == 0),
                    stop=(jt == njt - 1),
                )

        pend = []
        for grp in range(ngr):
            diag = grp >= ngr - 2
            for m2 in range(2):
                po = m2 * D
                # the two heads' S-matmuls contract 64 partitions each on
                # disjoint ranges -> they run on the two 64-row PE tiles
                # concurrently when adjacent in the queue.
                sT = sTp.tile([128, 2, QB], F32, tag="sT", name="sT")
                for m in range(2):
                    jt = grp * 2 + m
                    c0 = 128 * (jt - 4 * qb) if diag else 0
                    nc.tensor.matmul(
                        sT[:, m, c0:QB],
                        lhsT=ktile[po:po + D, jt * 128:(jt + 1) * 128],
                        rhs=qtile[po:po + D, qb * QB + c0:(qb + 1) * QB],
                        start=True,
                        stop=True,
                    )
                if diag:  # causal mask on the two 128-wide diagonal blocks
                    for m in range(2):
                        c0 = 128 * (grp * 2 + m - 4 * qb)
                        nc.vector.tensor_add(
                            sT[:, m, c0:c0 + 128], sT[:, m, c0:c0 + 128], mask_sb[:, :]
                        )
                pT = pTp.tile([128, 2, QB], F32R, tag="pT", name="pT")
                # one exp per head-group; on diag groups slice from m0's
                # first valid column -- the m1 columns [c0, c0+128) this
                # covers are stale psum that no PV matmul ever reads.
                ce = 128 * (grp * 2 - 4 * qb) if diag else 0
                nc.scalar.activation(pT[:, :, ce:QB], sT[:, :, ce:QB], AF.Exp)
                pend.append((m2, grp, pT))
            while len(pend) > 4:   # PV lags 2 g-cycles behind exp
                emit_omms(*pend.pop(0))
            if filler:             # spread filler over remaining g-cycles
                n = -(-len(filler) // (ngr - grp))
                for _ in range(n):
                    filler.pop(0)()
        for ent in pend:
            emit_omms(*ent)
        while filler:
            filler.pop(0)()
        # normalize: att^T = outT[0:D] * (1/Z), Z = outT[D]
        for m2 in range(2):
            po = m2 * D
            zrow = smallp.tile([1, QB], F32, tag="zrow", name="zrow")
            nc.vector.tensor_copy(zrow[:, :], oT[m2][D:D + 1, :])
            rz = smallp.tile([1, QB], F32, tag="rz", name="rz")
            nc.vector.reciprocal_approx_fast(out=rz[:, :], in_=zrow[:, :])
            zs = smallp.tile([D, QB], F32, tag="zs", name="zs")
            nc.gpsimd.partition_broadcast(zs[:, :], rz[:, :], channels=D)
            nc.vector.tensor_mul(
                attT_sb[pair][po:po + D, qb * QB:(qb + 1) * QB],
                oT[m2][0:D, :],
                zs[:, :],
            )

    # ---------------- the pipeline ----------------
    # fillers are scheduled as late as dependencies allow, to keep the PE
    # fed (warm) during the ACT-heavy later units.
    for c in qk_chunks(0, 0) + v_chunks(0):
        c()
    unit(0, 0, qk_chunks(1, 0))
    unit(1, 0, qk_chunks(0, 1))
    unit(0, 1, qk_chunks(1, 1) + v_chunks(1))
    unit(1, 1, qk_chunks(0, 2) + s6_chunks(0))
    unit(0, 2, qk_chunks(1, 2) + v_chunks(2))
    unit(1, 2, qk_chunks(0, 3) + v_chunks(3))
    unit(0, 3, qk_chunks(1, 3) + s6_chunks(1))
    unit(1, 3, s6_chunks(2))
    for c in s6_chunks(3):
        c()


def build_nc():
    from contextlib import ExitStack

    nc = bacc.Bacc("TRN2", target_bir_lowering=False)
    xT = nc.dram_tensor("xT", [C, T], BF16, kind="ExternalInput")
    wqk = nc.dram_tensor("wqk", [C, 2 * CS], BF16, kind="ExternalInput")
    wv = nc.dram_tensor("wv", [C, CS], BF16, kind="ExternalInput")
    bqk = nc.dram_tensor("bqk", [2 * CS, 1], F32, kind="ExternalInput")
    wp = nc.dram_tensor("wp", [CS, C], BF16, kind="ExternalInput")
    masks = nc.dram_tensor("masks", [128, 128], F32, kind="ExternalInput")
    yT = nc.dram_tensor("yT", [C, T], BF16, kind="ExternalOutput")
    with tile.TileContext(nc) as tc:
        with nc.allow_low_precision(reason="bf16 inputs / fp32r matmul; accumulation stays fp32 in PSUM"):
            with ExitStack() as ctx:
                _build_body(nc, tc, ctx, xT, wqk, wv, bqk, wp, masks, yT)
    nc.compile()
    return nc


def make_masks():
    r = np.arange(128)[:, None]
    c = np.arange(128)[None, :]
    return np.where(r <= c, np.float32(0.0), np.float32(NEG)).astype(np.float32)


def make_in_maps(x, W_qkv, b_qkv, W_proj):
    scale = np.float32(1.0 / np.sqrt(D))
    mask_h = make_masks()
    bf = ml_dtypes.bfloat16
    in_maps = []
    for i in range(NCORES):
        b, g = divmod(i, HPC)
        cs0 = g * CS
        wq = W_qkv[:, cs0:cs0 + CS] * scale
        wk = W_qkv[:, C + cs0:C + cs0 + CS]
        bq = b_qkv[cs0:cs0 + CS] * scale
        bk = b_qkv[C + cs0:C + cs0 + CS]
        in_maps.append({
            "xT": np.ascontiguousarray(x[b].T).astype(bf),
            "wqk": np.concatenate([wq, wk], axis=1).astype(bf),
            "wv": np.ascontiguousarray(W_qkv[:, 2 * C + cs0:2 * C + cs0 + CS]).astype(bf),
            "bqk": np.concatenate([bq, bk])[:, None].astype(np.float32),
            "wp": np.ascontiguousarray(W_proj[cs0:cs0 + CS, :]).astype(bf),
            "masks": mask_h,
        })
    return in_maps


_NC_CACHE = None


def _get_nc():
    global _NC_CACHE
    if _NC_CACHE is None:
        _NC_CACHE = build_nc()
    return _NC_CACHE


def gather(results, b_qkv, W_proj, b_proj):
    Y = np.zeros((B, T, C), np.float32)
    for i in range(NCORES):
        Y[i // HPC] += results[i]["yT"].T.astype(np.float32)
    Y += (b_qkv[2 * C:].astype(np.float32) @ W_proj.astype(np.float32)
          + b_proj.astype(np.float32))[None, None, :]
    return Y


def kernel(x, W_qkv, b_qkv, W_proj, b_proj):
    global LAST_RESULT
    x = np.asarray(x, np.float32)
    W_qkv = np.asarray(W_qkv, np.float32)
    b_qkv = np.asarray(b_qkv, np.float32)
    W_proj = np.asarray(W_proj, np.float32)
    b_proj = np.asarray(b_proj, np.float32)

    nc = _get_nc()
    in_maps = make_in_maps(x, W_qkv, b_qkv, W_proj)
    res = run_bass_kernel_spmd(nc, in_maps, list(range(NCORES)), trace=TRACE)
    LAST_RESULT = res
    if TRACE and res.exec_time_ns is not None:
        print(f"HW exec time: {res.exec_time_ns} ns")
    return gather(res.results, b_qkv, W_proj, b_proj)


# revision 11
# speedup vs baseline: 1.4138x; 1.0405x over previous
"""Causal multi-head attention (B=2, T=2048, C=1024, H=16, d=64) on 8 trn2 cores.

Sharding: core i -> (batch b = i//4, head group g = i%4, 4 heads/core).
Data parallel over B, tensor parallel over heads; the out-proj partial sums
(contraction over this core's 256 channels) are reduced on the host during
the gather step, along with b_proj and the analytically-folded V bias.

Device kernel works entirely in [feature, token] (transposed) layout so no
on-device transposes are needed.

Perf design is driven by the PE HAM clock gate (2.4 GHz only under sustained
busy; recurring idle re-throttles to 1.2 GHz): the whole kernel is ONE
software pipeline in which projection work and the out-proj are fed to the
PE as filler inside the attention units, so the PE never starves while ACT
(exp, the second-busiest engine) chases it:

  QK(pair, tc4):  Q^T,K^T 512-token block for one head pair, k-loop paced
      by the bf16 x/w DMA stream at the front of the kernel.
  V(tc4):         4 V t-tiles (natural layout, stage-4 lhsT, both pairs'
      channels at once), ones column appended (row 64 = softmax Z).
  unit(pair, qb): attention g-cycles of 2 j-tiles x 2 heads: S^T (the two
      heads' K=64 matmuls land on the two 64-row PE tiles and run
      CONCURRENTLY when adjacent), causal mask (DVE), one exp per
      head-group on ACT (sliced so stale-psum cols are never consumed),
      PV accumulation lagged 2 g-cycles, then att^T = outT[0:64] * (1/Z)
      -> bf16 via reciprocal_approx_fast + GPSIMD partition_broadcast.
  Units run in ASCENDING qb order so unit (pair, qb) only needs q/k/v
      blocks 0..qb -- this is what lets projection/attention interleave.
  S6(qb): y^T = Wp.T @ att^T (bf16) as late filler; host sums partials.

x / W_qkv / W_v / W_proj travel as bf16 (halves DMA; ~1e-3 rel err), scores
and P stay fp32(r) end-to-end, accumulation always fp32 in PSUM.
"""

import numpy as np
import ml_dtypes

import concourse.bass as bass
import concourse.mybir as mybir
from concourse import bacc
import concourse.tile as tile
from concourse.bass_utils import run_bass_kernel_spmd

B, T, C, H, D = 2, 2048, 1024, 16, 64
NCORES = 8
HPC = 4            # heads per core
CS = HPC * D       # 256 channels per core (per Q/K/V block)
KT = C // 128      # 8 contraction tiles for the projections
NT = T // 128      # 16 token tiles of 128
QB = 512           # query block (psum bank width in fp32)
NQB = T // QB      # 4 query blocks
NEG = -1e9

F32 = mybir.dt.float32
F32R = mybir.dt.float32r
BF16 = mybir.dt.bfloat16

TRACE = False
LAST_RESULT = None


def _build_body(nc, tc, ctx, xT, wqk, wv, bqk, wp, masks, yT):
    AF = mybir.ActivationFunctionType

    persist = ctx.enter_context(tc.tile_pool(name="persist", bufs=1))

    xT_sb = persist.tile([128, KT, T], BF16, tag="xT", name="xT_sb")
    wqk_sb = persist.tile([128, KT, 2 * CS], BF16, tag="wqk", name="wqk_sb")
    wv_sb = persist.tile([128, KT, CS], BF16, tag="wv", name="wv_sb")
    bqk_sb = persist.tile([128, 4], F32, tag="bqk", name="bqk_sb")
    wp_sb = persist.tile([128, 2, C], BF16, tag="wp", name="wp_sb")
    mask_sb = persist.tile([128, 128], F32, tag="mask", name="mask_sb")
    qT_sb = [persist.tile([128, T], F32R, tag=f"qT{i}", name=f"qT{i}") for i in range(2)]
    kT_sb = [persist.tile([128, T], F32R, tag=f"kT{i}", name=f"kT{i}") for i in range(2)]
    v_sb = [persist.tile([128, HPC, D + 1], F32R, tag=f"v{t}", name=f"v{t}") for t in range(NT)]
    attT_sb = [persist.tile([128, T], BF16, tag=f"attT{i}", name=f"attT{i}") for i in range(2)]

    # PSUM: sT 2x2 banks + oT 2 + fill (shared stage1/stage6) 2 = 8 banks
    sTp = ctx.enter_context(tc.tile_pool(name="sT", bufs=2, space="PSUM"))
    oTp = ctx.enter_context(tc.tile_pool(name="outT", bufs=2, space="PSUM"))
    fillp = ctx.enter_context(tc.tile_pool(name="fillp", bufs=2, space="PSUM"))
    pTp = ctx.enter_context(tc.tile_pool(name="pT", bufs=6))
    smallp = ctx.enter_context(tc.tile_pool(name="small", bufs=8))
    ysp = ctx.enter_context(tc.tile_pool(name="ystage", bufs=6))

    # DMA order = consumption order; x lands in token-quarter slices so the
    # first QK/V chunks (and with them the whole pipeline) start after ~2MB
    # instead of waiting for the full input.
    def dma_x(tc4):
        nc.sync.dma_start(
            out=xT_sb[:, :, tc4 * QB:(tc4 + 1) * QB],
            in_=xT[:, :, tc4 * QB:(tc4 + 1) * QB],
        )
    nc.sync.dma_start(out=wqk_sb[:, :, :], in_=wqk[:, :, :])
    dma_x(0)
    nc.sync.dma_start(out=wv_sb[:, :, :], in_=wv[:, :, :])
    nc.sync.dma_start(out=bqk_sb[:, :], in_=bqk[:, :])
    dma_x(1)
    nc.sync.dma_start(out=wp_sb[:, :, :], in_=wp[:, :, :])
    nc.sync.dma_start(out=mask_sb[:, :], in_=masks[:, :])
    dma_x(2)
    dma_x(3)

    ones_f32 = persist.tile([128, 4], F32, tag="ones_f32", name="ones_f32")
    nc.vector.memset(ones_f32[:, :], 1.0)
    for t in range(NT):
        nc.vector.tensor_copy(v_sb[t][:, :, D], ones_f32[:, :])

    # ---------------- projection chunks (filler) ----------------
    def qk_chunks(pair, tc4):
        def qk_chunk(ct, dst):
            def run():
                ps = fillp.tile([128, QB], F32, tag="fp", name="fp")
                for k in range(KT):
                    nc.tensor.matmul(
                        ps[:, :],
                        lhsT=wqk_sb[:, k, ct * 128:(ct + 1) * 128],
                        rhs=xT_sb[:, k, tc4 * QB:(tc4 + 1) * QB],
                        start=(k == 0),
                        stop=(k == KT - 1),
                    )
                nc.vector.tensor_scalar_add(
                    dst[:, tc4 * QB:(tc4 + 1) * QB], ps[:, :], bqk_sb[:, ct:ct + 1]
                )
            return run
        return [qk_chunk(pair, qT_sb[pair]), qk_chunk(2 + pair, kT_sb[pair])]

    def v_chunks(tc4):
        def v_chunk(t):
            def run():
                ps = fillp.tile([128, QB], F32, tag="fp", name="fp")
                for k in range(KT):
                    nc.tensor.matmul(
                        ps[:, 0:CS],
                        lhsT=xT_sb[:, k, t * 128:(t + 1) * 128],
                        rhs=wv_sb[:, k, :],
                        start=(k == 0),
                        stop=(k == KT - 1),
                    )
                nc.any.tensor_copy(
                    v_sb[t][:, :, 0:D],
                    ps[:, 0:CS].rearrange("p (h d) -> p h d", h=HPC),
                )
            return run
        return [v_chunk(t) for t in range(tc4 * 4, tc4 * 4 + 4)]

    # ---------------- stage 6 chunks (filler) ----------------
    def s6_chunks(sqb):
        def et_chunk(et):
            def run():
                yps_t = fillp.tile([128, QB], F32, tag="fp", name="fp")
                for kc in range(2):
                    nc.tensor.matmul(
                        yps_t[:, :],
                        lhsT=wp_sb[:, kc, et * 128:(et + 1) * 128],
                        rhs=attT_sb[kc][:, sqb * QB:(sqb + 1) * QB],
                        start=(kc == 0),
                        stop=(kc == 1),
                    )
                ys = ysp.tile([128, QB], BF16, tag="ys", name="ys")
                if et % 2 == 0:
                    nc.vector.tensor_copy(ys[:, :], yps_t[:, :])
                else:
                    nc.scalar.activation(ys[:, :], yps_t[:, :], AF.Copy)
                nc.sync.dma_start(
                    out=yT[et * 128:(et + 1) * 128, sqb * QB:(sqb + 1) * QB],
                    in_=ys[:, :],
                )
            return run
        return [et_chunk(et) for et in range(C // 128)]

    # ---------------- attention unit ----------------
    def unit(pair, qb, filler):
        ktile, qtile = kT_sb[pair], qT_sb[pair]
        ngr = 2 * (qb + 1)
        njt = 4 * (qb + 1)
        oT = [oTp.tile([128, QB], F32, tag="oT", name="oT") for _ in range(2)]

        def emit_omms(m2, grp, pT):
            h = pair * 2 + m2
            for m in range(2):
                jt = grp * 2 + m
                c0 = 128 * (jt - 4 * qb) if grp >= ngr - 2 else 0
                nc.tensor.matmul(
                    oT[m2][0:D + 1, c0:QB],
                    lhsT=v_sb[jt][:, h, :],
                    rhs=pT[:, m, c0:QB],
                    start=(jt == 0),
                    stop=(jt == njt - 1),
                )

        pend = []
        for grp in range(ngr):
            diag = grp >= ngr - 2
            for m2 in range(2):
                po = m2 * D
                # the two heads' S-matmuls contract 64 partitions each on
                # disjoint ranges -> they run on the two 64-row PE tiles
                # concurrently when adjacent in the queue.
                sT = sTp.tile([128, 2, QB], F32, tag="sT", name="sT")
                for m in range(2):
                    jt = grp * 2 + m
                    c0 = 128 * (jt - 4 * qb) if diag else 0
                    nc.tensor.matmul(
                        sT[:, m, c0:QB],
                        lhsT=ktile[po:po + D, jt * 128:(jt + 1) * 128],
                        rhs=qtile[po:po + D, qb * QB + c0:(qb + 1) * QB],
                        start=True,
                        stop=True,
                    )
                if diag:  # causal mask on the two 128-wide diagonal blocks
                    for m in range(2):
                        c0 = 128 * (grp * 2 + m - 4 * qb)
                        nc.vector.tensor_add(
                            sT[:, m, c0:c0 + 128], sT[:, m, c0:c0 + 128], mask_sb[:, :]
                        )
                pT = pTp.tile([128, 2, QB], F32R, tag="pT", name="pT")
                # one exp per head-group; on diag groups slice from m0's
                # first valid column -- the m1 columns [c0, c0+128) this
                # covers are stale psum that no PV matmul ever reads.
                ce = 128 * (grp * 2 - 4 * qb) if diag else 0
                nc.scalar.activation(pT[:, :, ce:QB], sT[:, :, ce:QB], AF.Exp)
                pend.append((m2, grp, pT))
            while len(pend) > 4:   # PV lags 2 g-cycles behind exp
                emit_omms(*pend.pop(0))
            if filler:             # spread filler over remaining g-cycles
                n = -(-len(filler) // (ngr - grp))
                for _ in range(n):
                    filler.pop(0)()
        for ent in pend:
            emit_omms(*ent)
        while filler:
            filler.pop(0)()
        # normalize: att^T = outT[0:D] * (1/Z), Z = outT[D]
        for m2 in range(2):
            po = m2 * D
            zrow = smallp.tile([1, QB], F32, tag="zrow", name="zrow")
            nc.vector.tensor_copy(zrow[:, :], oT[m2][D:D + 1, :])
            rz = smallp.tile([1, QB], F32, tag="rz", name="rz")
            nc.vector.reciprocal_approx_fast(out=rz[:, :], in_=zrow[:, :])
            zs = smallp.tile([D, QB], F32, tag="zs", name="zs")
            nc.gpsimd.partition_broadcast(zs[:, :], rz[:, :], channels=D)
            nc.vector.tensor_mul(
                attT_sb[pair][po:po + D, qb * QB:(qb + 1) * QB],
                oT[m2][0:D, :],
                zs[:, :],
            )

    # ---------------- the pipeline ----------------
    # fillers are scheduled as late as dependencies allow, to keep the PE
    # fed (warm) during the ACT-heavy later units.
    for c in qk_chunks(0, 0) + v_chunks(0):
        c()
    unit(0, 0, qk_chunks(1, 0))
    unit(1, 0, qk_chunks(0, 1))
    unit(0, 1, qk_chunks(1, 1) + v_chunks(1))
    unit(1, 1, qk_chunks(0, 2) + s6_chunks(0))
    unit(0, 2, qk_chunks(1, 2) + v_chunks(2))
    unit(1, 2, qk_chunks(0, 3) + v_chunks(3))
    unit(0, 3, qk_chunks(1, 3) + s6_chunks(1))
    unit(1, 3, s6_chunks(2))
    for c in s6_chunks(3):
        c()


def build_nc():
    from contextlib import ExitStack

    nc = bacc.Bacc("TRN2", target_bir_lowering=False)
    xT = nc.dram_tensor("xT", [128, KT, T], BF16, kind="ExternalInput")
    wqk = nc.dram_tensor("wqk", [128, KT, 2 * CS], BF16, kind="ExternalInput")
    wv = nc.dram_tensor("wv", [128, KT, CS], BF16, kind="ExternalInput")
    bqk = nc.dram_tensor("bqk", [128, 4], F32, kind="ExternalInput")
    wp = nc.dram_tensor("wp", [128, 2, C], BF16, kind="ExternalInput")
    masks = nc.dram_tensor("masks", [128, 128], F32, kind="ExternalInput")
    yT = nc.dram_tensor("yT", [C, T], BF16, kind="ExternalOutput")
    with tile.TileContext(nc) as tc:
        with nc.allow_low_precision(reason="bf16 inputs / fp32r matmul; accumulation stays fp32 in PSUM"):
            with ExitStack() as ctx:
                _build_body(nc, tc, ctx, xT, wqk, wv, bqk, wp, masks, yT)
    nc.compile()
    return nc


def make_masks():
    r = np.arange(128)[:, None]
    c = np.arange(128)[None, :]
    return np.where(r <= c, np.float32(0.0), np.float32(NEG)).astype(np.float32)


def make_in_maps(x, W_qkv, b_qkv, W_proj):
    scale = np.float32(1.0 / np.sqrt(D))
    mask_h = make_masks()
    bf = ml_dtypes.bfloat16
    in_maps = []
    for i in range(NCORES):
        b, g = divmod(i, HPC)
        cs0 = g * CS
        wq = W_qkv[:, cs0:cs0 + CS] * scale
        wk = W_qkv[:, C + cs0:C + cs0 + CS]
        bq = b_qkv[cs0:cs0 + CS] * scale
        bk = b_qkv[C + cs0:C + cs0 + CS]
        def ktiles(a):  # [K*128, N] -> [128, K, N]
            return np.ascontiguousarray(
                a.reshape(-1, 128, a.shape[1]).transpose(1, 0, 2)
            )
        in_maps.append({
            "xT": ktiles(x[b].T).astype(bf),
            "wqk": ktiles(np.concatenate([wq, wk], axis=1)).astype(bf),
            "wv": ktiles(np.ascontiguousarray(W_qkv[:, 2 * C + cs0:2 * C + cs0 + CS])).astype(bf),
            "bqk": np.ascontiguousarray(
                np.concatenate([bq, bk]).reshape(4, 128).T
            ).astype(np.float32),
            "wp": ktiles(np.ascontiguousarray(W_proj[cs0:cs0 + CS, :])).astype(bf),
            "masks": mask_h,
        })
    return in_maps


_NC_CACHE = None


def _get_nc():
    global _NC_CACHE
    if _NC_CACHE is None:
        _NC_CACHE = build_nc()
    return _NC_CACHE


def gather(results, b_qkv, W_proj, b_proj):
    Y = np.zeros((B, T, C), np.float32)
    for i in range(NCORES):
        Y[i // HPC] += results[i]["yT"].T.astype(np.float32)
    Y += (b_qkv[2 * C:].astype(np.float32) @ W_proj.astype(np.float32)
          + b_proj.astype(np.float32))[None, None, :]
    return Y


def kernel(x, W_qkv, b_qkv, W_proj, b_proj):
    global LAST_RESULT
    x = np.asarray(x, np.float32)
    W_qkv = np.asarray(W_qkv, np.float32)
    b_qkv = np.asarray(b_qkv, np.float32)
    W_proj = np.asarray(W_proj, np.float32)
    b_proj = np.asarray(b_proj, np.float32)

    nc = _get_nc()
    in_maps = make_in_maps(x, W_qkv, b_qkv, W_proj)
    res = run_bass_kernel_spmd(nc, in_maps, list(range(NCORES)), trace=TRACE)
    LAST_RESULT = res
    if TRACE and res.exec_time_ns is not None:
        print(f"HW exec time: {res.exec_time_ns} ns")
    return gather(res.results, b_qkv, W_proj, b_proj)


# revision 13
# speedup vs baseline: 1.4602x; 1.0328x over previous
"""Causal multi-head attention (B=2, T=2048, C=1024, H=16, d=64) on 8 trn2 cores.

Sharding: core i -> (batch b = i//4, head group g = i%4, 4 heads/core).
Data parallel over B, tensor parallel over heads; the out-proj partial sums
(contraction over this core's 256 channels) are reduced on the host during
the gather step, along with b_proj and the analytically-folded V bias.

Device kernel works entirely in [feature, token] (transposed) layout so no
on-device transposes are needed.

Perf design is driven by the PE HAM clock gate (2.4 GHz only under sustained
busy; recurring idle re-throttles to 1.2 GHz): the whole kernel is ONE
software pipeline in which projection work and the out-proj are fed to the
PE as filler inside the attention units, so the PE never starves while ACT
(exp, the second-busiest engine) chases it:

  QK(pair, tc4):  Q^T,K^T 512-token block for one head pair, k-loop paced
      by the bf16 x/w DMA stream at the front of the kernel.
  V(tc4):         4 V t-tiles (natural layout, stage-4 lhsT, both pairs'
      channels at once), ones column appended (row 64 = softmax Z).
  unit(pair, qb): attention g-cycles of 2 j-tiles x 2 heads: S^T (the two
      heads' K=64 matmuls land on the two 64-row PE tiles and run
      CONCURRENTLY when adjacent), causal mask (DVE), one exp per
      head-group on ACT (sliced so stale-psum cols are never consumed),
      PV accumulation lagged 2 g-cycles, then att^T = outT[0:64] * (1/Z)
      -> bf16 via reciprocal_approx_fast + GPSIMD partition_broadcast.
  Units run in ASCENDING qb order so unit (pair, qb) only needs q/k/v
      blocks 0..qb -- this is what lets projection/attention interleave.
  S6(qb): y^T = Wp.T @ att^T (bf16) as late filler; host sums partials.

x / W_qkv / W_v / W_proj travel as bf16 (halves DMA; ~1e-3 rel err), scores
and P stay fp32(r) end-to-end, accumulation always fp32 in PSUM.
"""

import numpy as np
import ml_dtypes

import concourse.bass as bass
import concourse.mybir as mybir
from concourse import bacc
import concourse.tile as tile
from concourse.bass_utils import run_bass_kernel_spmd

B, T, C, H, D = 2, 2048, 1024, 16, 64
NCORES = 8
HPC = 4            # heads per core
CS = HPC * D       # 256 channels per core (per Q/K/V block)
KT = C // 128      # 8 contraction tiles for the projections
NT = T // 128      # 16 token tiles of 128
QB = 512           # query block (psum bank width in fp32)
NQB = T // QB      # 4 query blocks
NEG = -1e9

F32 = mybir.dt.float32
F32R = mybir.dt.float32r
BF16 = mybir.dt.bfloat16

TRACE = False
LAST_RESULT = None


def _build_body(nc, tc, ctx, xT, wqk, wv, bqk, wp, masks, yT):
    AF = mybir.ActivationFunctionType

    persist = ctx.enter_context(tc.tile_pool(name="persist", bufs=1))

    xT_sb = persist.tile([128, KT, T], BF16, tag="xT", name="xT_sb")
    wqk_sb = persist.tile([128, KT, 2 * CS], BF16, tag="wqk", name="wqk_sb")
    wv_sb = persist.tile([128, KT, CS], BF16, tag="wv", name="wv_sb")
    bqk_sb = persist.tile([128, 4], F32, tag="bqk", name="bqk_sb")
    wp_sb = persist.tile([128, 2, C], BF16, tag="wp", name="wp_sb")
    mask_sb = persist.tile([128, 128], F32, tag="mask", name="mask_sb")
    qT_sb = [persist.tile([128, T], F32R, tag=f"qT{i}", name=f"qT{i}") for i in range(2)]
    kT_sb = [persist.tile([128, T], F32R, tag=f"kT{i}", name=f"kT{i}") for i in range(2)]
    v_sb = [persist.tile([128, HPC, D + 1], F32R, tag=f"v{t}", name=f"v{t}") for t in range(NT)]
    attT_sb = [persist.tile([128, T], BF16, tag=f"attT{i}", name=f"attT{i}") for i in range(2)]

    # PSUM: sT 2x2 banks + oT 2 + fill (shared stage1/stage6) 2 = 8 banks
    sTp = ctx.enter_context(tc.tile_pool(name="sT", bufs=2, space="PSUM"))
    oTp = ctx.enter_context(tc.tile_pool(name="outT", bufs=2, space="PSUM"))
    fillp = ctx.enter_context(tc.tile_pool(name="fillp", bufs=2, space="PSUM"))
    pTp = ctx.enter_context(tc.tile_pool(name="pT", bufs=6))
    smallp = ctx.enter_context(tc.tile_pool(name="small", bufs=8))
    ysp = ctx.enter_context(tc.tile_pool(name="ystage", bufs=6))

    # DMA order = consumption order; x lands in token-quarter slices so the
    # first QK/V chunks (and with them the whole pipeline) start after ~2MB
    # instead of waiting for the full input.
    def dma_x(tc4):
        nc.sync.dma_start(
            out=xT_sb[:, :, tc4 * QB:(tc4 + 1) * QB],
            in_=xT[:, :, tc4 * QB:(tc4 + 1) * QB],
        )
    nc.sync.dma_start(out=wqk_sb[:, 0:4, :], in_=wqk[:, 0:4, :])
    nc.sync.dma_start(
        out=xT_sb[:, 0:4, 0:QB], in_=xT[:, 0:4, 0:QB])
    nc.sync.dma_start(out=wqk_sb[:, 4:KT, :], in_=wqk[:, 4:KT, :])
    nc.sync.dma_start(
        out=xT_sb[:, 4:KT, 0:QB], in_=xT[:, 4:KT, 0:QB])
    nc.sync.dma_start(out=wv_sb[:, :, :], in_=wv[:, :, :])
    nc.sync.dma_start(out=bqk_sb[:, :], in_=bqk[:, :])
    dma_x(1)
    nc.sync.dma_start(out=wp_sb[:, :, :], in_=wp[:, :, :])
    nc.sync.dma_start(out=mask_sb[:, :], in_=masks[:, :])
    dma_x(2)
    dma_x(3)

    ones_f32 = persist.tile([128, 4], F32, tag="ones_f32", name="ones_f32")
    nc.vector.memset(ones_f32[:, :], 1.0)
    for t in range(NT):
        nc.vector.tensor_copy(v_sb[t][:, :, D], ones_f32[:, :])

    # ---------------- projection chunks (filler) ----------------
    def qk_chunks(pair, tc4):
        def qk_chunk(ct, dst):
            def run():
                ps = fillp.tile([128, QB], F32, tag="fp", name="fp")
                for k in range(KT):
                    nc.tensor.matmul(
                        ps[:, :],
                        lhsT=wqk_sb[:, k, ct * 128:(ct + 1) * 128],
                        rhs=xT_sb[:, k, tc4 * QB:(tc4 + 1) * QB],
                        start=(k == 0),
                        stop=(k == KT - 1),
                    )
                nc.vector.tensor_scalar_add(
                    dst[:, tc4 * QB:(tc4 + 1) * QB], ps[:, :], bqk_sb[:, ct:ct + 1]
                )
            return run
        return [qk_chunk(pair, qT_sb[pair]), qk_chunk(2 + pair, kT_sb[pair])]

    def v_chunks(tc4):
        def v_chunk(t):
            def run():
                ps = fillp.tile([128, QB], F32, tag="fp", name="fp")
                for k in range(KT):
                    nc.tensor.matmul(
                        ps[:, 0:CS],
                        lhsT=xT_sb[:, k, t * 128:(t + 1) * 128],
                        rhs=wv_sb[:, k, :],
                        start=(k == 0),
                        stop=(k == KT - 1),
                    )
                nc.any.tensor_copy(
                    v_sb[t][:, :, 0:D],
                    ps[:, 0:CS].rearrange("p (h d) -> p h d", h=HPC),
                )
            return run
        return [v_chunk(t) for t in range(tc4 * 4, tc4 * 4 + 4)]

    # ---------------- stage 6 chunks (filler) ----------------
    def s6_chunks(sqb):
        def et_chunk(e2):
            def run():
                ys = ysp.tile([128, 2, QB], BF16, tag="ys", name="ys")
                for sub in range(2):
                    et = e2 * 2 + sub
                    yps_t = fillp.tile([128, QB], F32, tag="fp", name="fp")
                    for kc in range(2):
                        nc.tensor.matmul(
                            yps_t[:, :],
                            lhsT=wp_sb[:, kc, et * 128:(et + 1) * 128],
                            rhs=attT_sb[kc][:, sqb * QB:(sqb + 1) * QB],
                            start=(kc == 0),
                            stop=(kc == 1),
                        )
                    if e2 % 2 == 0:
                        nc.vector.tensor_copy(ys[:, sub, :], yps_t[:, :])
                    else:
                        nc.scalar.activation(ys[:, sub, :], yps_t[:, :], AF.Copy)
                nc.sync.dma_start(
                    out=yT[:, e2 * 2:e2 * 2 + 2, sqb * QB:(sqb + 1) * QB],
                    in_=ys[:, :, :],
                )
            return run
        return [et_chunk(e2) for e2 in range(C // 256)]

    # ---------------- attention unit ----------------
    def unit(pair, qb, filler):
        ktile, qtile = kT_sb[pair], qT_sb[pair]
        ngr = 2 * (qb + 1)
        njt = 4 * (qb + 1)
        oT = [oTp.tile([128, QB], F32, tag="oT", name="oT") for _ in range(2)]

        def emit_omms(m2, grp, pT):
            h = pair * 2 + m2
            for m in range(2):
                jt = grp * 2 + m
                c0 = 128 * (jt - 4 * qb) if grp >= ngr - 2 else 0
                nc.tensor.matmul(
                    oT[m2][0:D + 1, c0:QB],
                    lhsT=v_sb[jt][:, h, :],
                    rhs=pT[:, m, c0:QB],
                    start=(jt == 0),
                    stop=(jt == njt - 1),
                )

        pend = []
        for grp in range(ngr):
            diag = grp >= ngr - 2
            for m2 in range(2):
                po = m2 * D
                # the two heads' S-matmuls contract 64 partitions each on
                # disjoint ranges -> they run on the two 64-row PE tiles
                # concurrently when adjacent in the queue.
                sT = sTp.tile([128, 2, QB], F32, tag="sT", name="sT")
                for m in range(2):
                    jt = grp * 2 + m
                    c0 = 128 * (jt - 4 * qb) if diag else 0
                    nc.tensor.matmul(
                        sT[:, m, c0:QB],
                        lhsT=ktile[po:po + D, jt * 128:(jt + 1) * 128],
                        rhs=qtile[po:po + D, qb * QB + c0:(qb + 1) * QB],
                        start=True,
                        stop=True,
                    )
                if diag:  # causal mask on the two 128-wide diagonal blocks
                    for m in range(2):
                        c0 = 128 * (grp * 2 + m - 4 * qb)
                        nc.vector.tensor_add(
                            sT[:, m, c0:c0 + 128], sT[:, m, c0:c0 + 128], mask_sb[:, :]
                        )
                pT = pTp.tile([128, 2, QB], F32R, tag="pT", name="pT")
                # one exp per head-group; on diag groups slice from m0's
                # first valid column -- the m1 columns [c0, c0+128) this
                # covers are stale psum that no PV matmul ever reads.
                ce = 128 * (grp * 2 - 4 * qb) if diag else 0
                nc.scalar.activation(pT[:, :, ce:QB], sT[:, :, ce:QB], AF.Exp)
                pend.append((m2, grp, pT))
            while len(pend) > 4:   # PV lags 2 g-cycles behind exp
                emit_omms(*pend.pop(0))
            if filler:             # spread filler over remaining g-cycles
                n = -(-len(filler) // (ngr - grp))
                for _ in range(n):
                    filler.pop(0)()
        for ent in pend:
            emit_omms(*ent)
        while filler:
            filler.pop(0)()
        # normalize: att^T = outT[0:D] * (1/Z), Z = outT[D]
        for m2 in range(2):
            po = m2 * D
            zrow = smallp.tile([1, QB], F32, tag="zrow", name="zrow")
            nc.vector.tensor_copy(zrow[:, :], oT[m2][D:D + 1, :])
            rz = smallp.tile([1, QB], F32, tag="rz", name="rz")
            nc.vector.reciprocal_approx_fast(out=rz[:, :], in_=zrow[:, :])
            zs = smallp.tile([D, QB], F32, tag="zs", name="zs")
            nc.gpsimd.partition_broadcast(zs[:, :], rz[:, :], channels=D)
            nc.vector.tensor_mul(
                attT_sb[pair][po:po + D, qb * QB:(qb + 1) * QB],
                oT[m2][0:D, :],
                zs[:, :],
            )

    # ---------------- the pipeline ----------------
    # fillers are scheduled as late as dependencies allow, to keep the PE
    # fed (warm) during the ACT-heavy later units.
    for c in qk_chunks(0, 0) + v_chunks(0):
        c()
    unit(0, 0, qk_chunks(1, 0))
    unit(1, 0, qk_chunks(0, 1))
    unit(0, 1, qk_chunks(1, 1) + v_chunks(1))
    unit(1, 1, qk_chunks(0, 2) + s6_chunks(0))
    unit(0, 2, qk_chunks(1, 2) + v_chunks(2))
    unit(1, 2, qk_chunks(0, 3) + v_chunks(3))
    unit(0, 3, qk_chunks(1, 3) + s6_chunks(1))
    unit(1, 3, s6_chunks(2))
    for c in s6_chunks(3):
        c()


def build_nc():
    from contextlib import ExitStack

    nc = bacc.Bacc("TRN2", target_bir_lowering=False)
    xT = nc.dram_tensor("xT", [128, KT, T], BF16, kind="ExternalInput")
    wqk = nc.dram_tensor("wqk", [128, KT, 2 * CS], BF16, kind="ExternalInput")
    wv = nc.dram_tensor("wv", [128, KT, CS], BF16, kind="ExternalInput")
    bqk = nc.dram_tensor("bqk", [128, 4], F32, kind="ExternalInput")
    wp = nc.dram_tensor("wp", [128, 2, C], BF16, kind="ExternalInput")
    masks = nc.dram_tensor("masks", [128, 128], F32, kind="ExternalInput")
    yT = nc.dram_tensor("yT", [128, C // 128, T], BF16, kind="ExternalOutput")
    with tile.TileContext(nc) as tc:
        with nc.allow_low_precision(reason="bf16 inputs / fp32r matmul; accumulation stays fp32 in PSUM"):
            with ExitStack() as ctx:
                _build_body(nc, tc, ctx, xT, wqk, wv, bqk, wp, masks, yT)
    nc.compile()
    return nc


def make_masks():
    r = np.arange(128)[:, None]
    c = np.arange(128)[None, :]
    return np.where(r <= c, np.float32(0.0), np.float32(NEG)).astype(np.float32)


def make_in_maps(x, W_qkv, b_qkv, W_proj):
    scale = np.float32(1.0 / np.sqrt(D))
    mask_h = make_masks()
    bf = ml_dtypes.bfloat16
    in_maps = []
    for i in range(NCORES):
        b, g = divmod(i, HPC)
        cs0 = g * CS
        wq = W_qkv[:, cs0:cs0 + CS] * scale
        wk = W_qkv[:, C + cs0:C + cs0 + CS]
        bq = b_qkv[cs0:cs0 + CS] * scale
        bk = b_qkv[C + cs0:C + cs0 + CS]
        def ktiles(a):  # [K*128, N] -> [128, K, N]
            return np.ascontiguousarray(
                a.reshape(-1, 128, a.shape[1]).transpose(1, 0, 2)
            )
        in_maps.append({
            "xT": ktiles(x[b].T).astype(bf),
            "wqk": ktiles(np.concatenate([wq, wk], axis=1)).astype(bf),
            "wv": ktiles(np.ascontiguousarray(W_qkv[:, 2 * C + cs0:2 * C + cs0 + CS])).astype(bf),
            "bqk": np.ascontiguousarray(
                np.concatenate([bq, bk]).reshape(4, 128).T
            ).astype(np.float32),
            "wp": ktiles(np.ascontiguousarray(W_proj[cs0:cs0 + CS, :])).astype(bf),
            "masks": mask_h,
        })
    return in_maps


_NC_CACHE = None


def _get_nc():
    global _NC_CACHE
    if _NC_CACHE is None:
        _NC_CACHE = build_nc()
    return _NC_CACHE


def gather(results, b_qkv, W_proj, b_proj):
    Y = np.zeros((B, T, C), np.float32)
    for i in range(NCORES):
        Y[i // HPC] += results[i]["yT"].transpose(1, 0, 2).reshape(C, T).T.astype(np.float32)
    Y += (b_qkv[2 * C:].astype(np.float32) @ W_proj.astype(np.float32)
          + b_proj.astype(np.float32))[None, None, :]
    return Y


def kernel(x, W_qkv, b_qkv, W_proj, b_proj):
    global LAST_RESULT
    x = np.asarray(x, np.float32)
    W_qkv = np.asarray(W_qkv, np.float32)
    b_qkv = np.asarray(b_qkv, np.float32)
    W_proj = np.asarray(W_proj, np.float32)
    b_proj = np.asarray(b_proj, np.float32)

    nc = _get_nc()
    in_maps = make_in_maps(x, W_qkv, b_qkv, W_proj)
    res = run_bass_kernel_spmd(nc, in_maps, list(range(NCORES)), trace=TRACE)
    LAST_RESULT = res
    if TRACE and res.exec_time_ns is not None:
        print(f"HW exec time: {res.exec_time_ns} ns")
    return gather(res.results, b_qkv, W_proj, b_proj)
